# revision 62
# baseline (speedup 1.0000x reference)
"""Trainium2 Bass kernel for nn_Attention_9096740733536 (sparse_attention).

Sharding: data-parallel over the QB (task) dim across 8 cores (2 tasks/core),
one mid-kernel AllReduce of [feat_corr partials | q_global | k_global] sums.
The attention math is algebraically collapsed: mixed scores are linear (no
softmax), so
  out[h,q] = alpha_h*(Fq/qn) @ ((Fk/kn)^T @ Fv) + ww_h*q_ratio (x) (k_ratio^T Fv)
with 128x128 inner matrices instead of 512x512 score matrices, and layernorm
is folded into the input projection via rank-1 PSUM augmentation.
"""
import numpy as np
from contextlib import ExitStack

import concourse.bass as bass
import concourse.tile as tile
from concourse import bacc, mybir
from concourse import bass_utils
from concourse._compat import with_exitstack

F32 = mybir.dt.float32
F32R = mybir.dt.float32r
AF = mybir.ActivationFunctionType
ALU = mybir.AluOpType
AX = mybir.AxisListType

H, D, DIM = 8, 128, 1024
QB, N = 16, 512
N_CORES = 8
T = QB * N // N_CORES          # 1024 tokens per core
NT = T // 128                  # 8 token tiles per core
NTASK = T // N                 # 2 tasks per core
LN_EPS = 1e-5
TOK_ALL = float(QB * N)


@with_exitstack
def attn_kernel(ctx: ExitStack, tc: tile.TileContext, outs, ins, n_cores=N_CORES):
    nc = tc.nc
    y = outs[0]
    (xn_q, xn_k, xn_v, xT_q, xT_k, xT_v, Wp_d, WoT_d, negu_d, vrow_d,
     bout_d, ones_d, ident_d, mask_d, wp1T_d, wp2T_d, b1_d, gbc_d, bbc_d,
     b2bc_d) = ins

    consts = ctx.enter_context(tc.tile_pool(name="consts", bufs=1))
    fpool = ctx.enter_context(tc.tile_pool(name="fpool", bufs=1))
    stat1 = ctx.enter_context(tc.tile_pool(name="stat1", bufs=1))
    dram = ctx.enter_context(tc.tile_pool(name="dram", bufs=1, space="DRAM"))

    ps_proj = ctx.enter_context(tc.tile_pool(name="ps_proj", bufs=3, space="PSUM"))
    ps_fc = ctx.enter_context(tc.tile_pool(name="ps_fc", bufs=2, space="PSUM"))
    ps_gk = ctx.enter_context(tc.tile_pool(name="ps_gk", bufs=1, space="PSUM"))
    ps_o1 = ctx.enter_context(tc.tile_pool(name="ps_o1", bufs=1, space="PSUM"))
    ps_small = ctx.enter_context(tc.tile_pool(name="ps_small", bufs=1, space="PSUM"))

    # ---- small constants (long-lived) ----
    ident = consts.tile([128, 128], F32)
    nc.sync.dma_start(ident[:], ident_d[:])
    bout = consts.tile([1, DIM], F32R)
    nc.sync.dma_start(bout[:], bout_d[:].bitcast(F32R))
    onesr = consts.tile([1, 128], F32R)
    nc.sync.dma_start(onesr[:], ones_d[0:1, :].bitcast(F32R))
    ones = consts.tile([128, 8], F32)
    nc.sync.dma_start(ones[:], ones_d[:, 0:8])
    mask_nd = consts.tile([128, H * 128], F32)
    nc.scalar.dma_start(mask_nd[:], mask_d[:])
    wp1T = consts.tile([128, 256], F32)
    nc.scalar.dma_start(wp1T[:], wp1T_d[:])
    wp2T = consts.tile([128, 3], F32)
    nc.scalar.dma_start(wp2T[:], wp2T_d[:])
    b1row = consts.tile([1, 128], F32)
    nc.scalar.dma_start(b1row[:], b1_d[:])
    ones8 = consts.tile([1, 8], F32)
    nc.sync.dma_start(ones8[:], ones_d[0:1, 0:8])
    gbc = consts.tile([8, 128], F32)
    nc.scalar.dma_start(gbc[:], gbc_d[:])
    bbc = consts.tile([8, 128], F32)
    nc.scalar.dma_start(bbc[:], bbc_d[:])
    b2bc = consts.tile([8, 3], F32)
    nc.scalar.dma_start(b2bc[:], b2bc_d[:])
    eps = consts.tile([128, 1], F32)
    nc.vector.memset(eps[:], LN_EPS)

    # ---- persistent F tensors: [128 tok, t*1024 + h*128 + d] ----
    Fq = fpool.tile([128, NT * DIM], F32)
    Fk = fpool.tile([128, NT * DIM], F32)
    Fv = fpool.tile([128, NT * DIM], F32)
    sq_scr = stat1.tile([128, DIM], F32)     # ACT square scratch (write-only)

    xns = [xn_q, xn_k, xn_v]
    xTs = [xT_q, xT_k, xT_v]
    Fs = [Fq, Fk, Fv]

    # ======== Phase 1: folded-LN projection (scoped pools) ========
    with tc.tile_pool(name="ph1", bufs=1) as ph1, \
         tc.tile_pool(name="xpool", bufs=3) as xpool, \
         tc.tile_pool(name="spool", bufs=3) as spool:
        Wp = ph1.tile([128, 8 * DIM], F32R)
        for s in range(8):
            nc.gpsimd.dma_start(Wp[:, s * DIM:(s + 1) * DIM],
                                Wp_d[:, s * DIM:(s + 1) * DIM].bitcast(F32R))
        negu = ph1.tile([1, DIM], F32R)
        nc.sync.dma_start(negu[:], negu_d[:].bitcast(F32R))
        vrow = ph1.tile([1, DIM], F32R)
        nc.sync.dma_start(vrow[:], vrow_d[:].bitcast(F32R))
        for t in range(NT):
            st = spool.tile([128, 12], F32, tag="st")
            bn6 = spool.tile([128, 36], F32, tag="bn6")
            rsig = spool.tile([128, 3], F32, tag="rsig")
            for i in range(3):
                xn = xpool.tile([128, DIM], F32, tag="xn")
                nc.sync.dma_start(xn[:], xns[i][t * 128:(t + 1) * 128, :])
                nc.vector.bn_stats(bn6[:, i * 12:i * 12 + 6], xn[:, 0:512])
                nc.vector.bn_stats(bn6[:, i * 12 + 6:i * 12 + 12],
                                   xn[:, 512:1024])
                # (mean, var) pair -> st cols (6+i, 9+i via sqrt)
                nc.vector.bn_aggr(st[:, 2 * i:2 * i + 2],
                                  bn6[:, i * 12:i * 12 + 12])
            # st cols 0,2,4 = mu ; 1,3,5 = var
            nc.vector.tensor_copy(st[:, 6:9], st[:, 0:6:2])
            nc.scalar.activation(st[:, 9:12], st[:, 1:6:2], AF.Sqrt,
                                 bias=eps[:])
            nc.vector.reciprocal(rsig[:], st[:, 9:12])
            # transpose [mu|sig] (cols 6..11) -> rows [6, 128] -> flat [1, 768]
            trp = ps_small.tile([6, 128], F32, tag="sm")
            nc.tensor.transpose(trp[:], st[:, 6:12], ident[:])
            rows6 = spool.tile([6, 128], F32R, tag="rows6")
            nc.scalar.copy(rows6[:], trp[:])
            rows = spool.tile([1, 768], F32R, tag="rows")
            nc.scalar.dma_start(rows[:], rows6[:])
            for i in range(3):
                xT_t = xpool.tile([128, DIM], F32R, tag="xT")
                nc.sync.dma_start(xT_t[:],
                                  xTs[i][:, t * DIM:(t + 1) * DIM].bitcast(F32R))
                for half in range(2):
                    o = half * 512
                    acc = ps_proj.tile([128, 512], F32, tag="proj")
                    for s in range(8):
                        nc.tensor.matmul(
                            acc[:], xT_t[:, s * 128:(s + 1) * 128],
                            Wp[:, s * DIM + o: s * DIM + o + 512],
                            start=(s == 0), stop=False)
                    nc.tensor.matmul(acc[:], rows[:, i * 128:(i + 1) * 128],
                                     negu[:, o:o + 512], start=False, stop=False)
                    nc.tensor.matmul(acc[:], rows[:, (3 + i) * 128:(4 + i) * 128],
                                     vrow[:, o:o + 512], start=False, stop=True)
                    dst = Fs[i][:, t * DIM + o: t * DIM + o + 512]
                    if (i + half) % 2 == 0:
                        nc.scalar.mul(dst, acc[:], rsig[:, i:i + 1])
                    else:
                        nc.vector.tensor_scalar_mul(dst, acc[:],
                                                    rsig[:, i:i + 1])

    # ======== Phase 2: F stats, feat_corr partials, q/k globals ========
    late = ctx.enter_context(tc.tile_pool(name="late", bufs=1))
    WoT = late.tile([128, 8 * DIM], F32R)
    nc.gpsimd.dma_start(WoT[:], WoT_d[:].bitcast(F32R))

    qss = stat1.tile([128, 64], F32)   # col t*8+h : sumsq over d of Fq
    qsm = stat1.tile([128, 64], F32)   # sums over d
    kss = stat1.tile([128, 64], F32)
    ksm = stat1.tile([128, 64], F32)
    qmean = stat1.tile([128, 64], F32)
    qninv = stat1.tile([128, 64], F32)
    kninv = stat1.tile([128, 64], F32)
    kn = stat1.tile([128, 64], F32)
    qr = stat1.tile([128, 64], F32)
    kr = stat1.tile([128, 64], F32)
    rscr = stat1.tile([128, 96], F32)  # ratio-chain scratch (3x32 per half)

    def derived(ss, sm, ninv, ratio, s, n_out=None):
        # ninv = 1/sqrt(ss); var = ss/127 - sm^2/(128*127)
        # ratio = 2*min(var,1)/(var+1)
        w = s.stop - s.start
        if n_out is not None:
            nc.scalar.activation(n_out[:, s], ss[:, s], AF.Sqrt)
            nc.vector.reciprocal(ninv[:, s], n_out[:, s])
        else:
            nc.scalar.activation(ninv[:, s], ss[:, s], AF.Sqrt)
            nc.vector.reciprocal(ninv[:, s], ninv[:, s])
        t1 = rscr[:, 0:w]
        nc.vector.tensor_tensor(t1, sm[:, s], sm[:, s], op=ALU.mult)
        nc.vector.tensor_scalar_mul(t1, t1, 1.0 / (D * (D - 1)))
        t2 = rscr[:, w:2 * w]
        nc.vector.tensor_scalar_mul(t2, ss[:, s], 1.0 / (D - 1))
        var = rscr[:, 2 * w:3 * w]
        nc.vector.tensor_tensor(var, t2, t1, op=ALU.subtract)
        nc.vector.tensor_scalar(t1, var, 1.0, 2.0, ALU.min, ALU.mult)
        nc.vector.tensor_scalar_add(t2, var, 1.0)
        nc.vector.reciprocal(t2, t2)
        nc.vector.tensor_tensor(ratio[:, s], t1, t2, op=ALU.mult)

    for jh in range(NTASK):
        for t in range(4 * jh, 4 * jh + 4):
            nc.vector.reduce_sum(
                qsm[:, t * 8:(t + 1) * 8],
                Fq[:, t * DIM:(t + 1) * DIM].rearrange("p (h d) -> p h d", h=8),
                axis=AX.X)
            nc.vector.reduce_sum(
                ksm[:, t * 8:(t + 1) * 8],
                Fk[:, t * DIM:(t + 1) * DIM].rearrange("p (h d) -> p h d", h=8),
                axis=AX.X)
            for h in range(H):
                sl = slice(t * DIM + h * 128, t * DIM + h * 128 + 128)
                nc.scalar.activation(sq_scr[:, 0:128], Fq[:, sl], AF.Square,
                                     accum_out=qss[:, t * 8 + h:t * 8 + h + 1])
                nc.scalar.activation(sq_scr[:, 128:256], Fk[:, sl], AF.Square,
                                     accum_out=kss[:, t * 8 + h:t * 8 + h + 1])
        s = slice(jh * 32, jh * 32 + 32)
        # NOTE: qmean holds NEGATED means (used as ACT bias for centering)
        nc.vector.tensor_scalar_mul(qmean[:, s], qsm[:, s], -1.0 / D)
        derived(qss, qsm, qninv, qr, s)
        derived(kss, ksm, kninv, kr, s, n_out=kn)
        # absorb kn into k_ratio: mv uses scaled Fv, so kr must carry kn back
        nc.vector.tensor_tensor(kr[:, s], kr[:, s], kn[:, s], op=ALU.mult)
        # scale Fv in place by 1/kn (only consumer is the M/mv stage)
        for t in range(4 * jh, 4 * jh + 4):
            for h in range(H):
                sl = slice(t * DIM + h * 128, t * DIM + h * 128 + 128)
                nc.vector.tensor_scalar(Fv[:, sl], Fv[:, sl],
                                        kninv[:, t * 8 + h:t * 8 + h + 1],
                                        None, ALU.mult)

    # ======== Phase 4a: allreduce-independent M/mv stage ========
    # M = Fk^T @ (Fv/kn) and mv = (kr*kn)^T @ (Fv/kn) per (head, task),
    # evicted UNSCALED (alpha/ww applied post-allreduce). Placed BEFORE the
    # feat_corr stage so the in-order PE stream overlaps the phase-1 tail.
    attn = ctx.enter_context(tc.tile_pool(name="attn", bufs=1))
    mm_raw = {}
    mv_raw = {}
    for j in range(NTASK):
        for h in range(H):
            mm_ps = ps_fc.tile([128, 128], F32, tag="fc128", name="mm_ps")
            mv_ps = ps_small.tile([1, 128], F32, tag="sm", name="mv_ps")
            for ti in range(4):
                t = 4 * j + ti
                sl = slice(t * DIM + h * 128, t * DIM + h * 128 + 128)
                nc.tensor.matmul(mm_ps[:], Fk[:, sl], Fv[:, sl],
                                 start=(ti == 0), stop=(ti == 3))
                nc.tensor.matmul(mv_ps[:], kr[:, t * 8 + h:t * 8 + h + 1],
                                 Fv[:, sl], start=(ti == 0), stop=(ti == 3))
            mm = attn.tile([128, 128], F32R, tag=f"mm{h}{j}", name="mm")
            nc.scalar.copy(mm[:], mm_ps[:])
            mv = attn.tile([1, 128], F32R, tag=f"mv{h}{j}", name="mv")
            nc.scalar.copy(mv[:], mv_ps[:])
            mm_raw[(h, j)] = mm
            mv_raw[(h, j)] = mv

    # feat_corr partials (per head) + q/k global sums (single PSUM group)
    # t-outer emission so no engine stream blocks on the last proj tile.
    ar_in = dram.tile([128, H * 128 + 16], F32)
    ar_out = dram.tile([128, H * 128 + 16], F32)
    gk_ps = ps_gk.tile([128, 16], F32, tag="gk")
    with tc.tile_pool(name="ph2", bufs=2) as ph2, \
         tc.tile_pool(name="qcpool", bufs=64) as qcpool:
        qc_tiles = {}
        for t in range(NT):
            for h in range(H):
                sl = slice(t * DIM + h * 128, t * DIM + h * 128 + 128)
                qc = qcpool.tile([128, 128], mybir.dt.bfloat16, tag="qc",
                                 name="qc")
                nc.scalar.activation(qc[:], Fq[:, sl], AF.Identity,
                                     bias=qmean[:, t * 8 + h:t * 8 + h + 1])
                qc_tiles[(t, h)] = qc
                first = (h == 0 and t == 0)
                last = (h == H - 1 and t == NT - 1)
                nc.tensor.matmul(gk_ps[:, h:h + 1], Fq[:, sl], ones[:, 0:1],
                                 start=first, stop=last, skip_group_check=True)
                nc.tensor.matmul(gk_ps[:, 8 + h:9 + h], Fk[:, sl], ones[:, 0:1],
                                 start=False, stop=False, skip_group_check=True)
        for h in range(H):
            fc_ps = ps_fc.tile([128, 128], F32, tag="fc128", name="fc_ps")
            for t in range(NT):
                nc.tensor.matmul(fc_ps[:], qc_tiles[(t, h)][:],
                                 qc_tiles[(t, h)][:],
                                 start=(t == 0), stop=(t == NT - 1))
            fc_sb = ph2.tile([128, 128], F32, tag="fcsb", name="fc_sb")
            nc.scalar.copy(fc_sb[:], fc_ps[:])
            nc.sync.dma_start(ar_in[:, h * 128:(h + 1) * 128], fc_sb[:])
        gk_sb = ph2.tile([128, 16], F32, tag="gksb", name="gk_sb")
        nc.scalar.copy(gk_sb[:], gk_ps[:])
        nc.sync.dma_start(ar_in[:, H * 128:H * 128 + 16], gk_sb[:])

    # in-place Fq <- Fq/qn (after feat_corr reads; gates only phase 4b)
    for h in range(H):
        for t in range(NT):
            sl = slice(t * DIM + h * 128, t * DIM + h * 128 + 128)
            c = slice(t * 8 + h, t * 8 + h + 1)
            nc.vector.tensor_scalar(Fq[:, sl], Fq[:, sl], qninv[:, c], None,
                                    ALU.mult)

    # ======== AllReduce ========
    if n_cores > 1:
        nc.gpsimd.collective_compute(
            "AllReduce", ALU.add,
            replica_groups=[list(range(n_cores))],
            ins=[ar_in.opt()], outs=[ar_out.opt()])
    else:  # single-core sim variant: allreduce over one core == copy
        nc.sync.dma_start(ar_out[:], ar_in[:])
    ar = late.tile([128, H * 128 + 16], F32)
    nc.sync.dma_start(ar[:], ar_out[:])
    arg = ar[:, H * 128:H * 128 + 16]

    # ======== Phase 3: decorr scale + weight predictor ========
    ssq = stat1.tile([128, 8], F32)
    msk = late.tile([128, H * 128], F32)
    nc.vector.tensor_tensor(msk[:], ar[:, 0:H * 128], mask_nd[:], op=ALU.mult)
    nc.scalar.activation(sq_scr[:, 0:H * 128], msk[:], AF.Square,
                         scale=1.0 / TOK_ALL)
    nc.vector.reduce_sum(ssq[:],
                         sq_scr[:, 0:H * 128].rearrange("p (h d) -> p h d", h=8),
                         axis=AX.X)
    ss_ps = ps_small.tile([8, 8], F32, tag="sm", name="ss_ps")
    nc.tensor.matmul(ss_ps[:], ssq[:], ones[:, 0:8], start=True, stop=True)
    dsc = stat1.tile([8, 8], F32)
    nc.scalar.activation(dsc[:, 0:1], ss_ps[0:8, 0:1], AF.Sqrt)
    nc.scalar.activation(dsc[:, 1:2], dsc[:, 0:1], AF.Exp, scale=-5.0 / (D * D))

    featsq = stat1.tile([128, 8], F32)
    nc.vector.tensor_scalar_mul(featsq[:], arg[:, 0:8], 1.0 / TOK_ALL)
    featsk = stat1.tile([128, 8], F32)
    nc.vector.tensor_scalar_mul(featsk[:], arg[:, 8:16], 1.0 / TOK_ALL)
    h1_ps = ps_small.tile([8, 128], F32, tag="sm", name="h1_ps")
    nc.tensor.matmul(h1_ps[:], featsq[:], wp1T[:, 0:128], start=True, stop=False)
    nc.tensor.matmul(h1_ps[:], featsk[:], wp1T[:, 128:256], start=False,
                     stop=False)
    nc.tensor.matmul(h1_ps[:], ones8[:], b1row[:], start=False, stop=True)
    h1 = stat1.tile([8, 128], F32)
    nc.scalar.copy(h1[:], h1_ps[:])
    w_mu = stat1.tile([8, 4], F32)
    nc.vector.reduce_sum(w_mu[:, 0:1], h1[:], axis=AX.X)
    nc.vector.tensor_scalar_mul(w_mu[:, 0:1], w_mu[:, 0:1], 1.0 / D)
    nc.scalar.activation(sq_scr[0:8, 0:128], h1[:], AF.Square,
                         accum_out=w_mu[:, 1:2])
    nc.vector.tensor_scalar_mul(w_mu[:, 1:2], w_mu[:, 1:2], 1.0 / D)
    nc.vector.tensor_tensor(w_mu[:, 2:3], w_mu[:, 0:1], w_mu[:, 0:1], op=ALU.mult)
    nc.vector.tensor_tensor(w_mu[:, 2:3], w_mu[:, 1:2], w_mu[:, 2:3],
                            op=ALU.subtract)
    nc.scalar.activation(w_mu[:, 3:4], w_mu[:, 2:3], AF.Sqrt, bias=eps[0:8, :])
    nc.vector.reciprocal(w_mu[:, 3:4], w_mu[:, 3:4])
    h1n = stat1.tile([8, 128], F32)
    nc.vector.tensor_scalar(h1n[:], h1[:], w_mu[:, 0:1], w_mu[:, 3:4],
                            ALU.subtract, ALU.mult)
    nc.vector.tensor_tensor(h1n[:], h1n[:], gbc[:], op=ALU.mult)
    nc.vector.tensor_tensor(h1n[:], h1n[:], bbc[:], op=ALU.add)
    nc.vector.tensor_scalar_max(h1n[:], h1n[:], 0.0)
    h1T_ps = ps_small.tile([128, 8], F32, tag="sm", name="h1T_ps")
    nc.tensor.transpose(h1T_ps[:], h1n[:], ident[0:8, 0:8])
    h1T = stat1.tile([128, 8], F32)
    nc.scalar.copy(h1T[:], h1T_ps[:])
    lg_ps = ps_small.tile([8, 3], F32, tag="sm", name="lg_ps")
    nc.tensor.matmul(lg_ps[:], h1T[:], wp2T[:], start=True, stop=True)
    lg = stat1.tile([8, 8], F32)
    nc.scalar.copy(lg[:, 0:3], lg_ps[:])
    nc.vector.tensor_tensor(lg[:, 0:3], lg[:, 0:3], b2bc[:], op=ALU.add)
    # logits are O(1): skip the (mathematically redundant) max-subtraction
    nc.scalar.activation(lg[:, 0:3], lg[:, 0:3], AF.Exp)
    nc.vector.reduce_sum(lg[:, 4:5], lg[:, 0:3], axis=AX.X)
    nc.vector.reciprocal(lg[:, 4:5], lg[:, 4:5])
    nc.vector.tensor_scalar(lg[:, 0:3], lg[:, 0:3], lg[:, 4:5], None, ALU.mult)
    # alpha = w0 + w1*dsc ; ww = w2 ; broadcast to 128 partitions
    aw = stat1.tile([8, 2], F32)
    nc.vector.tensor_tensor(aw[:, 0:1], lg[:, 1:2], dsc[:, 1:2], op=ALU.mult)
    nc.vector.tensor_tensor(aw[:, 0:1], aw[:, 0:1], lg[:, 0:1], op=ALU.add)
    nc.vector.tensor_copy(aw[:, 1:2], lg[:, 2:3])
    awT_ps = ps_small.tile([2, 8], F32, tag="sm", name="awT_ps")
    nc.tensor.transpose(awT_ps[:], aw[:], ident[0:8, 0:8])
    awT = stat1.tile([2, 8], F32)
    nc.scalar.copy(awT[:], awT_ps[:])
    aw_flat = stat1.tile([1, 16], F32)
    nc.scalar.dma_start(aw_flat[:], awT[:])
    abc = stat1.tile([128, 8], F32)
    nc.gpsimd.partition_broadcast(abc[:], aw_flat[:, 0:8])
    wbc = stat1.tile([128, 8], F32)
    nc.gpsimd.partition_broadcast(wbc[:], aw_flat[:, 8:16])

    # ======== Phase 4b + 5: scaled attention + output projection ========
    with tc.tile_pool(name="ph4", bufs=2) as ph4, \
         tc.tile_pool(name="o1pool", bufs=10) as o1pool:
        o1_tiles = {}
        for j in range(NTASK):
            for h in range(H):
                mm_sb = ph4.tile([128, 128], F32R, tag="mmsb", name="mm_sb")
                nc.vector.tensor_scalar(mm_sb[:], mm_raw[(h, j)][:],
                                        abc[:, h:h + 1], None, ALU.mult)
                mv_sb = ph4.tile([1, 128], F32R, tag="mvsb", name="mv_sb")
                nc.vector.tensor_scalar(mv_sb[:], mv_raw[(h, j)][:],
                                        wbc[0:1, h:h + 1], None, ALU.mult)

                # q_ratio row for this (h, j): [1, 512]
                c0 = 4 * j * 8 + h
                wq_ps = ps_small.tile([4, 128], F32, tag="sm", name="wq_ps")
                nc.tensor.transpose(wq_ps[:], qr[:, c0:c0 + 25:8], ident[:])
                wq4 = ph4.tile([4, 128], F32R, tag="wq4", name="wq4")
                nc.scalar.copy(wq4[:], wq_ps[:])
                wqr = ph4.tile([1, 512], F32R, tag="wqr", name="wqr")
                nc.scalar.dma_start(wqr[:], wq4[:])

                fqTs = ph4.tile([128, 512], F32R, tag="fqTs", name="fqTs")
                for ti in range(4):
                    t = 4 * j + ti
                    sl = slice(t * DIM + h * 128, t * DIM + h * 128 + 128)
                    qsT_ps = ps_fc.tile([128, 128], F32, tag="fc128",
                                        name="qsT_ps")
                    nc.tensor.transpose(qsT_ps[:], Fq[:, sl], ident[:])
                    nc.scalar.copy(fqTs[:, ti * 128:(ti + 1) * 128], qsT_ps[:])

                o1_ps = ps_o1.tile([128, 512], F32, tag="o1", name="o1_ps")
                nc.tensor.matmul(o1_ps[:], mm_sb[:], fqTs[:], start=True,
                                 stop=False)
                nc.tensor.matmul(o1_ps[:], mv_sb[:], wqr[:],
                                 start=False, stop=True)
                o1 = o1pool.tile([128, 512], F32R, tag="o1sb", name="o1_sb")
                nc.scalar.copy(o1[:], o1_ps[:])
                o1_tiles[(h, j)] = o1

            # ---- output projection for this task ----
            for t in range(4 * j, 4 * j + 4):
                ti = t % 4
                for half in range(2):
                    o = half * 512
                    op_ps = ps_proj.tile([128, 512], F32, tag="proj",
                                         name="op_ps")
                    for h in range(H):
                        nc.tensor.matmul(
                            op_ps[:],
                            o1_tiles[(h, j)][:, ti * 128:(ti + 1) * 128],
                            WoT[:, h * DIM + o: h * DIM + o + 512],
                            start=(h == 0), stop=False)
                    nc.tensor.matmul(op_ps[:], onesr[:, 0:128],
                                     bout[:, o:o + 512],
                                     start=False, stop=True)
                    ysb = ph4.tile([128, 512], F32, tag="ysb", name="ysb")
                    nc.scalar.copy(ysb[:], op_ps[:])
                    nc.sync.dma_start(y[t * 128:(t + 1) * 128, o:o + 512],
                                      ysb[:])


_BUILT = {}


def _build(n_cores=N_CORES):
    if n_cores in _BUILT:
        return _BUILT[n_cores]
    nc = bacc.Bacc("TRN2", target_bir_lowering=False, debug=False,
                   num_devices=n_cores)
    in_specs = [
        ("xn_q", [T, DIM]), ("xn_k", [T, DIM]), ("xn_v", [T, DIM]),
        ("xT_q", [128, NT * DIM]), ("xT_k", [128, NT * DIM]),
        ("xT_v", [128, NT * DIM]),
        ("Wp", [128, 8 * DIM]), ("WoT", [128, 8 * DIM]),
        ("negu", [1, DIM]), ("vrow", [1, DIM]), ("bout", [1, DIM]),
        ("ones", [128, 128]), ("ident", [128, 128]), ("mask", [128, 1024]),
        ("wp1T", [128, 256]), ("wp2T", [128, 3]), ("b1row", [1, 128]),
        ("gbc", [8, 128]), ("bbc", [8, 128]), ("b2bc", [8, 3]),
    ]
    in_aps = [nc.dram_tensor(n, s, F32, kind="ExternalInput").ap()
              for n, s in in_specs]
    y_ap = nc.dram_tensor("y", [T, DIM], F32, kind="ExternalOutput").ap()
    with tile.TileContext(nc) as tc:
        attn_kernel(tc, [y_ap], in_aps, n_cores=n_cores)
    nc.compile()
    _BUILT[n_cores] = nc
    return nc


def kernel(q, k, v, ln_g, ln_b, w_in, wp_w1, wp_b1, wp_ln_g, wp_ln_b,
           wp_w2, wp_b2, w_out, b_out):
    q = np.asarray(q, dtype=np.float32)
    k = np.asarray(k, dtype=np.float32)
    v = np.asarray(v, dtype=np.float32)
    ln_g = np.asarray(ln_g, np.float32); ln_b = np.asarray(ln_b, np.float32)
    w_in = np.asarray(w_in, np.float32); w_out = np.asarray(w_out, np.float32)
    b_out = np.asarray(b_out, np.float32)
    wp_w1 = np.asarray(wp_w1, np.float32); wp_b1 = np.asarray(wp_b1, np.float32)
    wp_ln_g = np.asarray(wp_ln_g, np.float32)
    wp_ln_b = np.asarray(wp_ln_b, np.float32)
    wp_w2 = np.asarray(wp_w2, np.float32); wp_b2 = np.asarray(wp_b2, np.float32)

    # host weight prep (folded layernorm)
    W = w_in.T                                     # [DIM, HD]
    Wp = (ln_g[:, None] * W)
    negu = -(ln_g @ W)[None, :]
    vrow = (ln_b @ W)[None, :]
    Wp_t = np.ascontiguousarray(
        Wp.reshape(8, 128, 2, 512).transpose(1, 0, 2, 3)).reshape(128, -1)
    WoT = np.ascontiguousarray(
        w_out.T.reshape(8, 128, DIM).transpose(1, 0, 2)).reshape(128, -1)
    shared = {
        "Wp": Wp_t, "WoT": WoT, "negu": negu, "vrow": vrow,
        "bout": b_out[None, :],
        "ones": np.ones((128, 128), np.float32),
        "ident": np.eye(128, dtype=np.float32),
        "mask": np.tile((1.0 - np.eye(128)).astype(np.float32), (1, 8)),
        "wp1T": np.ascontiguousarray(wp_w1.T.reshape(2, 128, 128)
                                     .transpose(1, 0, 2)).reshape(128, 256),
        "wp2T": np.ascontiguousarray(wp_w2.T),
        "b1row": wp_b1[None, :],
        "gbc": np.tile(wp_ln_g[None, :], (8, 1)),
        "bbc": np.tile(wp_ln_b[None, :], (8, 1)),
        "b2bc": np.tile(wp_b2[None, :], (8, 1)),
    }
    shared = {kk: np.ascontiguousarray(vv, np.float32)
              for kk, vv in shared.items()}

    qf = q.reshape(QB * N, DIM)
    kf = k.reshape(QB * N, DIM)
    vf = v.reshape(QB * N, DIM)
    in_maps = []
    for c in range(N_CORES):
        sl = slice(c * T, (c + 1) * T)
        m = dict(shared)
        for nm, arr in (("q", qf[sl]), ("k", kf[sl]), ("v", vf[sl])):
            m[f"xn_{nm}"] = np.ascontiguousarray(arr)
            m[f"xT_{nm}"] = np.ascontiguousarray(
                arr.reshape(NT, 128, 8, 128).transpose(3, 0, 2, 1)
            ).reshape(128, NT * DIM)
        in_maps.append(m)

    nc = _build()
    res = bass_utils.run_bass_kernel_spmd(nc, in_maps,
                                          core_ids=list(range(N_CORES)))
    global LAST_RESULTS
    LAST_RESULTS = res
    out = np.concatenate([r["y"] for r in res.results], axis=0)
    return out.reshape(QB, N, DIM)


LAST_RESULTS = None


# revision 81
# speedup vs baseline: 1.0393x; 1.0393x over previous
"""Trainium2 Bass kernel for nn_Attention_9096740733536 (sparse_attention).

Sharding: data-parallel over the QB (task) dim across 8 cores (2 tasks/core),
one mid-kernel AllReduce of [feat_corr partials | q_global | k_global] sums.
The attention math is algebraically collapsed: mixed scores are linear (no
softmax), so
  out[h,q] = alpha_h*(Fq/qn) @ ((Fk/kn)^T @ Fv) + ww_h*q_ratio (x) (k_ratio^T Fv)
with 128x128 inner matrices instead of 512x512 score matrices, and layernorm
is folded into the input projection via rank-1 PSUM augmentation.
"""
import numpy as np
from contextlib import ExitStack

import concourse.bass as bass
import concourse.tile as tile
from concourse import bacc, mybir
from concourse import bass_utils
from concourse._compat import with_exitstack

F32 = mybir.dt.float32
F32R = mybir.dt.float32r
AF = mybir.ActivationFunctionType
ALU = mybir.AluOpType
AX = mybir.AxisListType

H, D, DIM = 8, 128, 1024
QB, N = 16, 512
N_CORES = 8
T = QB * N // N_CORES          # 1024 tokens per core
NT = T // 128                  # 8 token tiles per core
NTASK = T // N                 # 2 tasks per core
LN_EPS = 1e-5
TOK_ALL = float(QB * N)


@with_exitstack
def attn_kernel(ctx: ExitStack, tc: tile.TileContext, outs, ins, n_cores=N_CORES):
    nc = tc.nc
    y = outs[0]
    (xn_q, xn_k, xn_v, xT_q, xT_k, xT_v, Wp_d, WoT_d, negu_d, vrow_d,
     bout_d, ones_d, ident_d, mask_d, wp1T_d, wp2T_d, b1_d, gbc_d, bbc_d,
     b2bc_d) = ins

    consts = ctx.enter_context(tc.tile_pool(name="consts", bufs=1))
    fpool = ctx.enter_context(tc.tile_pool(name="fpool", bufs=1))
    stat1 = ctx.enter_context(tc.tile_pool(name="stat1", bufs=1))
    dram = ctx.enter_context(tc.tile_pool(name="dram", bufs=1, space="DRAM"))

    ps_proj = ctx.enter_context(tc.tile_pool(name="ps_proj", bufs=3, space="PSUM"))
    ps_fc = ctx.enter_context(tc.tile_pool(name="ps_fc", bufs=2, space="PSUM"))
    ps_gk = ctx.enter_context(tc.tile_pool(name="ps_gk", bufs=1, space="PSUM"))
    ps_o1 = ctx.enter_context(tc.tile_pool(name="ps_o1", bufs=1, space="PSUM"))
    ps_small = ctx.enter_context(tc.tile_pool(name="ps_small", bufs=1, space="PSUM"))

    # ---- small constants (long-lived) ----
    ident = consts.tile([128, 128], F32)
    nc.sync.dma_start(ident[:], ident_d[:])
    bout = consts.tile([1, DIM], F32R)
    nc.sync.dma_start(bout[:], bout_d[:].bitcast(F32R))
    onesr = consts.tile([1, 128], F32R)
    nc.sync.dma_start(onesr[:], ones_d[0:1, :].bitcast(F32R))
    ones = consts.tile([128, 8], F32)
    nc.sync.dma_start(ones[:], ones_d[:, 0:8])
    mask_nd = consts.tile([128, H * 128], F32)
    nc.scalar.dma_start(mask_nd[:], mask_d[:])
    wp1T = consts.tile([128, 256], F32)
    nc.scalar.dma_start(wp1T[:], wp1T_d[:])
    wp2T = consts.tile([128, 3], F32)
    nc.scalar.dma_start(wp2T[:], wp2T_d[:])
    b1row = consts.tile([1, 128], F32)
    nc.scalar.dma_start(b1row[:], b1_d[:])
    ones8 = consts.tile([1, 8], F32)
    nc.sync.dma_start(ones8[:], ones_d[0:1, 0:8])
    gbc = consts.tile([8, 128], F32)
    nc.scalar.dma_start(gbc[:], gbc_d[:])
    bbc = consts.tile([8, 128], F32)
    nc.scalar.dma_start(bbc[:], bbc_d[:])
    b2bc = consts.tile([8, 3], F32)
    nc.scalar.dma_start(b2bc[:], b2bc_d[:])
    eps = consts.tile([128, 1], F32)
    nc.vector.memset(eps[:], LN_EPS)

    # ---- persistent F tensors: [128 tok, t*1024 + h*128 + d] ----
    Fq = fpool.tile([128, NT * DIM], F32)
    Fk = fpool.tile([128, NT * DIM], F32)
    Fv = fpool.tile([128, NT * DIM], F32)
    sq_scr = stat1.tile([128, DIM], F32)     # ACT square scratch (write-only)

    xns = [xn_q, xn_k, xn_v]
    xTs = [xT_q, xT_k, xT_v]
    Fs = [Fq, Fk, Fv]

    # ======== Phase 1: folded-LN projection (scoped pools) ========
    with tc.tile_pool(name="ph1", bufs=1) as ph1, \
         tc.tile_pool(name="xpool", bufs=3) as xpool, \
         tc.tile_pool(name="spool", bufs=3) as spool:
        Wp = ph1.tile([128, 8 * DIM], F32R)
        for s in range(8):
            nc.gpsimd.dma_start(Wp[:, s * DIM:(s + 1) * DIM],
                                Wp_d[:, s * DIM:(s + 1) * DIM].bitcast(F32R))
        negu = ph1.tile([1, DIM], F32R)
        nc.sync.dma_start(negu[:], negu_d[:].bitcast(F32R))
        vrow = ph1.tile([1, DIM], F32R)
        nc.sync.dma_start(vrow[:], vrow_d[:].bitcast(F32R))
        for t in range(NT):
            st = spool.tile([128, 12], F32, tag="st")
            bn6 = spool.tile([128, 36], F32, tag="bn6")
            rsig = spool.tile([128, 3], F32, tag="rsig")
            for i in range(3):
                xn = xpool.tile([128, DIM], F32, tag="xn")
                nc.sync.dma_start(xn[:], xns[i][t * 128:(t + 1) * 128, :])
                nc.vector.bn_stats(bn6[:, i * 12:i * 12 + 6], xn[:, 0:512])
                nc.vector.bn_stats(bn6[:, i * 12 + 6:i * 12 + 12],
                                   xn[:, 512:1024])
                # (mean, var) pair -> st cols (6+i, 9+i via sqrt)
                nc.vector.bn_aggr(st[:, 2 * i:2 * i + 2],
                                  bn6[:, i * 12:i * 12 + 12])
            # st cols 0,2,4 = mu ; 1,3,5 = var
            nc.vector.tensor_copy(st[:, 6:9], st[:, 0:6:2])
            nc.scalar.activation(st[:, 9:12], st[:, 1:6:2], AF.Sqrt,
                                 bias=eps[:])
            nc.vector.reciprocal(rsig[:], st[:, 9:12])
            # transpose [mu|sig] (cols 6..11) -> rows [6, 128] -> flat [1, 768]
            trp = ps_small.tile([6, 128], F32, tag="sm")
            nc.tensor.transpose(trp[:], st[:, 6:12], ident[:])
            rows6 = spool.tile([6, 128], F32R, tag="rows6")
            nc.scalar.copy(rows6[:], trp[:])
            rows = spool.tile([1, 768], F32R, tag="rows")
            nc.scalar.dma_start(rows[:], rows6[:])
            for i in range(3):
                xT_t = xpool.tile([128, DIM], F32R, tag="xT")
                nc.sync.dma_start(xT_t[:],
                                  xTs[i][:, t * DIM:(t + 1) * DIM].bitcast(F32R))
                for half in range(2):
                    o = half * 512
                    acc = ps_proj.tile([128, 512], F32, tag="proj")
                    for s in range(8):
                        nc.tensor.matmul(
                            acc[:], xT_t[:, s * 128:(s + 1) * 128],
                            Wp[:, s * DIM + o: s * DIM + o + 512],
                            start=(s == 0), stop=False)
                    nc.tensor.matmul(acc[:], rows[:, i * 128:(i + 1) * 128],
                                     negu[:, o:o + 512], start=False, stop=False)
                    nc.tensor.matmul(acc[:], rows[:, (3 + i) * 128:(4 + i) * 128],
                                     vrow[:, o:o + 512], start=False, stop=True)
                    dst = Fs[i][:, t * DIM + o: t * DIM + o + 512]
                    if (i + half) % 2 == 0:
                        nc.scalar.mul(dst, acc[:], rsig[:, i:i + 1])
                    else:
                        nc.vector.tensor_scalar_mul(dst, acc[:],
                                                    rsig[:, i:i + 1])

    # ======== Phase 2: F stats, feat_corr partials, q/k globals ========
    late = ctx.enter_context(tc.tile_pool(name="late", bufs=1))
    WoT = late.tile([128, 8 * DIM], F32R)
    nc.gpsimd.dma_start(WoT[:], WoT_d[:].bitcast(F32R))

    qss = stat1.tile([128, 64], F32)   # col t*8+h : sumsq over d of Fq
    qsm = stat1.tile([128, 64], F32)   # sums over d
    kss = stat1.tile([128, 64], F32)
    ksm = stat1.tile([128, 64], F32)
    qmean = stat1.tile([128, 64], F32)
    qninv = stat1.tile([128, 64], F32)
    kninv = stat1.tile([128, 64], F32)
    kn = stat1.tile([128, 64], F32)
    qr = stat1.tile([128, 64], F32)
    kr = stat1.tile([128, 64], F32)
    rscr = stat1.tile([128, 96], F32)  # ratio-chain scratch (3x32 per half)

    def derived(ss, sm, ninv, ratio, s, n_out=None):
        # ninv = 1/sqrt(ss); var = ss/127 - sm^2/(128*127)
        # ratio = 2*min(var,1)/(var+1)
        w = s.stop - s.start
        if n_out is not None:
            nc.scalar.activation(n_out[:, s], ss[:, s], AF.Sqrt)
            nc.vector.reciprocal(ninv[:, s], n_out[:, s])
        else:
            nc.scalar.activation(ninv[:, s], ss[:, s], AF.Sqrt)
            nc.vector.reciprocal(ninv[:, s], ninv[:, s])
        t1 = rscr[:, 0:w]
        nc.vector.tensor_tensor(t1, sm[:, s], sm[:, s], op=ALU.mult)
        nc.vector.tensor_scalar_mul(t1, t1, 1.0 / (D * (D - 1)))
        t2 = rscr[:, w:2 * w]
        nc.vector.tensor_scalar_mul(t2, ss[:, s], 1.0 / (D - 1))
        var = rscr[:, 2 * w:3 * w]
        nc.vector.tensor_tensor(var, t2, t1, op=ALU.subtract)
        nc.vector.tensor_scalar(t1, var, 1.0, 2.0, ALU.min, ALU.mult)
        nc.vector.tensor_scalar_add(t2, var, 1.0)
        nc.vector.reciprocal(t2, t2)
        nc.vector.tensor_tensor(ratio[:, s], t1, t2, op=ALU.mult)

    for jh in range(NTASK):
        for t in range(4 * jh, 4 * jh + 4):
            nc.vector.reduce_sum(
                qsm[:, t * 8:(t + 1) * 8],
                Fq[:, t * DIM:(t + 1) * DIM].rearrange("p (h d) -> p h d", h=8),
                axis=AX.X)
            nc.vector.reduce_sum(
                ksm[:, t * 8:(t + 1) * 8],
                Fk[:, t * DIM:(t + 1) * DIM].rearrange("p (h d) -> p h d", h=8),
                axis=AX.X)
            for h in range(H):
                sl = slice(t * DIM + h * 128, t * DIM + h * 128 + 128)
                nc.scalar.activation(sq_scr[:, 0:128], Fq[:, sl], AF.Square,
                                     accum_out=qss[:, t * 8 + h:t * 8 + h + 1])
                nc.scalar.activation(sq_scr[:, 128:256], Fk[:, sl], AF.Square,
                                     accum_out=kss[:, t * 8 + h:t * 8 + h + 1])
        s = slice(jh * 32, jh * 32 + 32)
        # NOTE: qmean holds NEGATED means (used as ACT bias for centering)
        nc.vector.tensor_scalar_mul(qmean[:, s], qsm[:, s], -1.0 / D)
        derived(qss, qsm, qninv, qr, s)
        derived(kss, ksm, kninv, kr, s, n_out=kn)
        # absorb kn into k_ratio: mv uses scaled Fv, so kr must carry kn back
        nc.vector.tensor_tensor(kr[:, s], kr[:, s], kn[:, s], op=ALU.mult)
        # scale Fv in place by 1/kn (only consumer is the M/mv stage)
        for t in range(4 * jh, 4 * jh + 4):
            for h in range(H):
                sl = slice(t * DIM + h * 128, t * DIM + h * 128 + 128)
                nc.vector.tensor_scalar(Fv[:, sl], Fv[:, sl],
                                        kninv[:, t * 8 + h:t * 8 + h + 1],
                                        None, ALU.mult)

    # ======== Phase 4a: allreduce-independent M/mv stage ========
    # M = Fk^T @ (Fv/kn) and mv = (kr*kn)^T @ (Fv/kn) per (head, task),
    # evicted UNSCALED (alpha/ww applied post-allreduce). Placed BEFORE the
    # feat_corr stage so the in-order PE stream overlaps the phase-1 tail.
    attn = ctx.enter_context(tc.tile_pool(name="attn", bufs=1))
    mm_raw = {}
    mv_raw = {}
    for j in range(NTASK):
        for h in range(H):
            mm_ps = ps_fc.tile([128, 128], F32, tag="fc128", name="mm_ps")
            mv_ps = ps_small.tile([1, 128], F32, tag="sm", name="mv_ps")
            for ti in range(4):
                t = 4 * j + ti
                sl = slice(t * DIM + h * 128, t * DIM + h * 128 + 128)
                nc.tensor.matmul(mm_ps[:], Fk[:, sl], Fv[:, sl],
                                 start=(ti == 0), stop=(ti == 3))
                nc.tensor.matmul(mv_ps[:], kr[:, t * 8 + h:t * 8 + h + 1],
                                 Fv[:, sl], start=(ti == 0), stop=(ti == 3))
            mm = attn.tile([128, 128], F32R, tag=f"mm{h}{j}", name="mm")
            nc.scalar.copy(mm[:], mm_ps[:])
            mv = attn.tile([1, 128], F32R, tag=f"mv{h}{j}", name="mv")
            nc.scalar.copy(mv[:], mv_ps[:])
            mm_raw[(h, j)] = mm
            mv_raw[(h, j)] = mv

    # feat_corr partials (per head) + q/k global sums (single PSUM group)
    # t-outer emission so no engine stream blocks on the last proj tile.
    ar_in = dram.tile([128, H * 128 + 16], F32)
    ar_out = dram.tile([128, H * 128 + 16], F32)
    gk_ps = ps_gk.tile([128, 16], F32, tag="gk")
    with tc.tile_pool(name="ph2", bufs=2) as ph2, \
         tc.tile_pool(name="qcpool", bufs=64) as qcpool:
        qc_tiles = {}
        for t in range(NT):
            for h in range(H):
                sl = slice(t * DIM + h * 128, t * DIM + h * 128 + 128)
                qc = qcpool.tile([128, 128], mybir.dt.bfloat16, tag="qc",
                                 name="qc")
                nc.scalar.activation(qc[:], Fq[:, sl], AF.Identity,
                                     bias=qmean[:, t * 8 + h:t * 8 + h + 1])
                qc_tiles[(t, h)] = qc
                first = (h == 0 and t == 0)
                last = (h == H - 1 and t == NT - 1)
                nc.tensor.matmul(gk_ps[:, h:h + 1], Fq[:, sl], ones[:, 0:1],
                                 start=first, stop=last, skip_group_check=True)
                nc.tensor.matmul(gk_ps[:, 8 + h:9 + h], Fk[:, sl], ones[:, 0:1],
                                 start=False, stop=False, skip_group_check=True)
        for h in range(H):
            fc_ps = ps_fc.tile([128, 128], F32, tag="fc128", name="fc_ps")
            for t in range(NT):
                nc.tensor.matmul(fc_ps[:], qc_tiles[(t, h)][:],
                                 qc_tiles[(t, h)][:],
                                 start=(t == 0), stop=(t == NT - 1))
            fc_sb = ph2.tile([128, 128], F32, tag="fcsb", name="fc_sb")
            nc.vector.tensor_copy(fc_sb[:], fc_ps[:])
            nc.sync.dma_start(ar_in[:, h * 128:(h + 1) * 128], fc_sb[:])
        gk_sb = ph2.tile([128, 16], F32, tag="gksb", name="gk_sb")
        nc.scalar.copy(gk_sb[:], gk_ps[:])
        nc.sync.dma_start(ar_in[:, H * 128:H * 128 + 16], gk_sb[:])

    # in-place Fq <- Fq/qn (after feat_corr reads; gates only phase 4b)
    for h in range(H):
        for t in range(NT):
            sl = slice(t * DIM + h * 128, t * DIM + h * 128 + 128)
            c = slice(t * 8 + h, t * 8 + h + 1)
            nc.vector.tensor_scalar(Fq[:, sl], Fq[:, sl], qninv[:, c], None,
                                    ALU.mult)

    # ======== AllReduce ========
    if n_cores > 1:
        nc.gpsimd.collective_compute(
            "AllReduce", ALU.add,
            replica_groups=[list(range(n_cores))],
            ins=[ar_in.opt()], outs=[ar_out.opt()])
    else:  # single-core sim variant: allreduce over one core == copy
        nc.sync.dma_start(ar_out[:], ar_in[:])
    ar = late.tile([128, H * 128 + 16], F32)
    nc.sync.dma_start(ar[:], ar_out[:])
    arg = ar[:, H * 128:H * 128 + 16]

    # ======== Phase 3: decorr scale + weight predictor ========
    ssq = stat1.tile([128, 8], F32)
    msk = late.tile([128, H * 128], F32)
    nc.vector.tensor_tensor(msk[:], ar[:, 0:H * 128], mask_nd[:], op=ALU.mult)
    nc.scalar.activation(sq_scr[:, 0:H * 128], msk[:], AF.Square,
                         scale=1.0 / TOK_ALL)
    nc.vector.reduce_sum(ssq[:],
                         sq_scr[:, 0:H * 128].rearrange("p (h d) -> p h d", h=8),
                         axis=AX.X)
    ss_ps = ps_small.tile([8, 8], F32, tag="sm", name="ss_ps")
    nc.tensor.matmul(ss_ps[:], ssq[:], ones[:, 0:8], start=True, stop=True)
    dsc = stat1.tile([8, 8], F32)
    nc.scalar.activation(dsc[:, 0:1], ss_ps[0:8, 0:1], AF.Sqrt)
    nc.scalar.activation(dsc[:, 1:2], dsc[:, 0:1], AF.Exp, scale=-5.0 / (D * D))

    featsq = stat1.tile([128, 8], F32)
    nc.vector.tensor_scalar_mul(featsq[:], arg[:, 0:8], 1.0 / TOK_ALL)
    featsk = stat1.tile([128, 8], F32)
    nc.vector.tensor_scalar_mul(featsk[:], arg[:, 8:16], 1.0 / TOK_ALL)
    h1_ps = ps_small.tile([8, 128], F32, tag="sm", name="h1_ps")
    nc.tensor.matmul(h1_ps[:], featsq[:], wp1T[:, 0:128], start=True, stop=False)
    nc.tensor.matmul(h1_ps[:], featsk[:], wp1T[:, 128:256], start=False,
                     stop=False)
    nc.tensor.matmul(h1_ps[:], ones8[:], b1row[:], start=False, stop=True)
    h1 = stat1.tile([8, 128], F32)
    nc.scalar.copy(h1[:], h1_ps[:])
    w_mu = stat1.tile([8, 4], F32)
    nc.vector.reduce_sum(w_mu[:, 0:1], h1[:], axis=AX.X)
    nc.vector.tensor_scalar_mul(w_mu[:, 0:1], w_mu[:, 0:1], 1.0 / D)
    nc.scalar.activation(sq_scr[0:8, 0:128], h1[:], AF.Square,
                         accum_out=w_mu[:, 1:2])
    nc.vector.tensor_scalar_mul(w_mu[:, 1:2], w_mu[:, 1:2], 1.0 / D)
    nc.vector.tensor_tensor(w_mu[:, 2:3], w_mu[:, 0:1], w_mu[:, 0:1], op=ALU.mult)
    nc.vector.tensor_tensor(w_mu[:, 2:3], w_mu[:, 1:2], w_mu[:, 2:3],
                            op=ALU.subtract)
    nc.scalar.activation(w_mu[:, 3:4], w_mu[:, 2:3], AF.Sqrt, bias=eps[0:8, :])
    nc.vector.reciprocal(w_mu[:, 3:4], w_mu[:, 3:4])
    h1n = stat1.tile([8, 128], F32)
    nc.vector.tensor_scalar(h1n[:], h1[:], w_mu[:, 0:1], w_mu[:, 3:4],
                            ALU.subtract, ALU.mult)
    nc.vector.tensor_tensor(h1n[:], h1n[:], gbc[:], op=ALU.mult)
    nc.vector.tensor_tensor(h1n[:], h1n[:], bbc[:], op=ALU.add)
    nc.vector.tensor_scalar_max(h1n[:], h1n[:], 0.0)
    h1T_ps = ps_small.tile([128, 8], F32, tag="sm", name="h1T_ps")
    nc.tensor.transpose(h1T_ps[:], h1n[:], ident[0:8, 0:8])
    h1T = stat1.tile([128, 8], F32)
    nc.scalar.copy(h1T[:], h1T_ps[:])
    lg_ps = ps_small.tile([8, 3], F32, tag="sm", name="lg_ps")
    nc.tensor.matmul(lg_ps[:], h1T[:], wp2T[:], start=True, stop=True)
    lg = stat1.tile([8, 8], F32)
    nc.scalar.copy(lg[:, 0:3], lg_ps[:])
    nc.vector.tensor_tensor(lg[:, 0:3], lg[:, 0:3], b2bc[:], op=ALU.add)
    # logits are O(1): skip the (mathematically redundant) max-subtraction
    nc.scalar.activation(lg[:, 0:3], lg[:, 0:3], AF.Exp)
    nc.vector.reduce_sum(lg[:, 4:5], lg[:, 0:3], axis=AX.X)
    nc.vector.reciprocal(lg[:, 4:5], lg[:, 4:5])
    nc.vector.tensor_scalar(lg[:, 0:3], lg[:, 0:3], lg[:, 4:5], None, ALU.mult)
    # alpha = w0 + w1*dsc ; ww = w2 ; broadcast to 128 partitions
    aw = stat1.tile([8, 2], F32)
    nc.vector.tensor_tensor(aw[:, 0:1], lg[:, 1:2], dsc[:, 1:2], op=ALU.mult)
    nc.vector.tensor_tensor(aw[:, 0:1], aw[:, 0:1], lg[:, 0:1], op=ALU.add)
    nc.vector.tensor_copy(aw[:, 1:2], lg[:, 2:3])
    awT_ps = ps_small.tile([2, 8], F32, tag="sm", name="awT_ps")
    nc.tensor.transpose(awT_ps[:], aw[:], ident[0:8, 0:8])
    awT = stat1.tile([2, 8], F32)
    nc.scalar.copy(awT[:], awT_ps[:])
    aw_flat = stat1.tile([1, 16], F32)
    nc.scalar.dma_start(aw_flat[:], awT[:])
    abc = stat1.tile([128, 8], F32)
    nc.gpsimd.partition_broadcast(abc[:], aw_flat[:, 0:8])
    wbc = stat1.tile([128, 8], F32)
    nc.gpsimd.partition_broadcast(wbc[:], aw_flat[:, 8:16])

    # ======== Phase 4b + 5: scaled attention + output projection ========
    with tc.tile_pool(name="ph4", bufs=2) as ph4, \
         tc.tile_pool(name="o1pool", bufs=10) as o1pool:
        o1_tiles = {}
        for j in range(NTASK):
            for h in range(H):
                mm_sb = ph4.tile([128, 128], F32R, tag="mmsb", name="mm_sb")
                nc.vector.tensor_scalar(mm_sb[:], mm_raw[(h, j)][:],
                                        abc[:, h:h + 1], None, ALU.mult)
                mv_sb = ph4.tile([1, 128], F32R, tag="mvsb", name="mv_sb")
                nc.vector.tensor_scalar(mv_sb[:], mv_raw[(h, j)][:],
                                        wbc[0:1, h:h + 1], None, ALU.mult)

                # q_ratio row for this (h, j): [1, 512]
                c0 = 4 * j * 8 + h
                wq_ps = ps_small.tile([4, 128], F32, tag="sm", name="wq_ps")
                nc.tensor.transpose(wq_ps[:], qr[:, c0:c0 + 25:8], ident[:])
                wq4 = ph4.tile([4, 128], F32R, tag="wq4", name="wq4")
                nc.scalar.copy(wq4[:], wq_ps[:])
                wqr = ph4.tile([1, 512], F32R, tag="wqr", name="wqr")
                nc.scalar.dma_start(wqr[:], wq4[:])

                fqTs = ph4.tile([128, 512], F32R, tag="fqTs", name="fqTs")
                for ti in range(4):
                    t = 4 * j + ti
                    sl = slice(t * DIM + h * 128, t * DIM + h * 128 + 128)
                    qsT_ps = ps_fc.tile([128, 128], F32, tag="fc128",
                                        name="qsT_ps")
                    nc.tensor.transpose(qsT_ps[:], Fq[:, sl], ident[:])
                    nc.scalar.copy(fqTs[:, ti * 128:(ti + 1) * 128], qsT_ps[:])

                o1_ps = ps_o1.tile([128, 512], F32, tag="o1", name="o1_ps")
                nc.tensor.matmul(o1_ps[:], mm_sb[:], fqTs[:], start=True,
                                 stop=False)
                nc.tensor.matmul(o1_ps[:], mv_sb[:], wqr[:],
                                 start=False, stop=True)
                o1 = o1pool.tile([128, 512], F32R, tag="o1sb", name="o1_sb")
                nc.vector.tensor_copy(o1[:], o1_ps[:])
                o1_tiles[(h, j)] = o1

            # ---- output projection for this task ----
            for t in range(4 * j, 4 * j + 4):
                ti = t % 4
                for half in range(2):
                    o = half * 512
                    op_ps = ps_proj.tile([128, 512], F32, tag="proj",
                                         name="op_ps")
                    for h in range(H):
                        nc.tensor.matmul(
                            op_ps[:],
                            o1_tiles[(h, j)][:, ti * 128:(ti + 1) * 128],
                            WoT[:, h * DIM + o: h * DIM + o + 512],
                            start=(h == 0), stop=False)
                    nc.tensor.matmul(op_ps[:], onesr[:, 0:128],
                                     bout[:, o:o + 512],
                                     start=False, stop=True)
                    ysb = ph4.tile([128, 512], F32, tag="ysb", name="ysb")
                    nc.vector.tensor_copy(ysb[:], op_ps[:])
                    nc.sync.dma_start(y[t * 128:(t + 1) * 128, o:o + 512],
                                      ysb[:])


_BUILT = {}


def _build(n_cores=N_CORES):
    if n_cores in _BUILT:
        return _BUILT[n_cores]
    nc = bacc.Bacc("TRN2", target_bir_lowering=False, debug=False,
                   num_devices=n_cores)
    in_specs = [
        ("xn_q", [T, DIM]), ("xn_k", [T, DIM]), ("xn_v", [T, DIM]),
        ("xT_q", [128, NT * DIM]), ("xT_k", [128, NT * DIM]),
        ("xT_v", [128, NT * DIM]),
        ("Wp", [128, 8 * DIM]), ("WoT", [128, 8 * DIM]),
        ("negu", [1, DIM]), ("vrow", [1, DIM]), ("bout", [1, DIM]),
        ("ones", [128, 128]), ("ident", [128, 128]), ("mask", [128, 1024]),
        ("wp1T", [128, 256]), ("wp2T", [128, 3]), ("b1row", [1, 128]),
        ("gbc", [8, 128]), ("bbc", [8, 128]), ("b2bc", [8, 3]),
    ]
    in_aps = [nc.dram_tensor(n, s, F32, kind="ExternalInput").ap()
              for n, s in in_specs]
    y_ap = nc.dram_tensor("y", [T, DIM], F32, kind="ExternalOutput").ap()
    with tile.TileContext(nc) as tc:
        attn_kernel(tc, [y_ap], in_aps, n_cores=n_cores)
    nc.compile()
    _BUILT[n_cores] = nc
    return nc


def kernel(q, k, v, ln_g, ln_b, w_in, wp_w1, wp_b1, wp_ln_g, wp_ln_b,
           wp_w2, wp_b2, w_out, b_out):
    q = np.asarray(q, dtype=np.float32)
    k = np.asarray(k, dtype=np.float32)
    v = np.asarray(v, dtype=np.float32)
    ln_g = np.asarray(ln_g, np.float32); ln_b = np.asarray(ln_b, np.float32)
    w_in = np.asarray(w_in, np.float32); w_out = np.asarray(w_out, np.float32)
    b_out = np.asarray(b_out, np.float32)
    wp_w1 = np.asarray(wp_w1, np.float32); wp_b1 = np.asarray(wp_b1, np.float32)
    wp_ln_g = np.asarray(wp_ln_g, np.float32)
    wp_ln_b = np.asarray(wp_ln_b, np.float32)
    wp_w2 = np.asarray(wp_w2, np.float32); wp_b2 = np.asarray(wp_b2, np.float32)

    # host weight prep (folded layernorm)
    W = w_in.T                                     # [DIM, HD]
    Wp = (ln_g[:, None] * W)
    negu = -(ln_g @ W)[None, :]
    vrow = (ln_b @ W)[None, :]
    Wp_t = np.ascontiguousarray(
        Wp.reshape(8, 128, 2, 512).transpose(1, 0, 2, 3)).reshape(128, -1)
    WoT = np.ascontiguousarray(
        w_out.T.reshape(8, 128, DIM).transpose(1, 0, 2)).reshape(128, -1)
    shared = {
        "Wp": Wp_t, "WoT": WoT, "negu": negu, "vrow": vrow,
        "bout": b_out[None, :],
        "ones": np.ones((128, 128), np.float32),
        "ident": np.eye(128, dtype=np.float32),
        "mask": np.tile((1.0 - np.eye(128)).astype(np.float32), (1, 8)),
        "wp1T": np.ascontiguousarray(wp_w1.T.reshape(2, 128, 128)
                                     .transpose(1, 0, 2)).reshape(128, 256),
        "wp2T": np.ascontiguousarray(wp_w2.T),
        "b1row": wp_b1[None, :],
        "gbc": np.tile(wp_ln_g[None, :], (8, 1)),
        "bbc": np.tile(wp_ln_b[None, :], (8, 1)),
        "b2bc": np.tile(wp_b2[None, :], (8, 1)),
    }
    shared = {kk: np.ascontiguousarray(vv, np.float32)
              for kk, vv in shared.items()}

    qf = q.reshape(QB * N, DIM)
    kf = k.reshape(QB * N, DIM)
    vf = v.reshape(QB * N, DIM)
    in_maps = []
    for c in range(N_CORES):
        sl = slice(c * T, (c + 1) * T)
        m = dict(shared)
        for nm, arr in (("q", qf[sl]), ("k", kf[sl]), ("v", vf[sl])):
            m[f"xn_{nm}"] = np.ascontiguousarray(arr)
            m[f"xT_{nm}"] = np.ascontiguousarray(
                arr.reshape(NT, 128, 8, 128).transpose(3, 0, 2, 1)
            ).reshape(128, NT * DIM)
        in_maps.append(m)

    nc = _build()
    res = bass_utils.run_bass_kernel_spmd(nc, in_maps,
                                          core_ids=list(range(N_CORES)))
    global LAST_RESULTS
    LAST_RESULTS = res
    out = np.concatenate([r["y"] for r in res.results], axis=0)
    return out.reshape(QB, N, DIM)


LAST_RESULTS = None
